# revision 51
# baseline (speedup 1.0000x reference)
"""Trainium2 Bass kernel for nn_Atten_Block (non-local attention block).

Reference computation per batch element b (C=256, C4=64, H=W=64, N=4096):
    theta = W1 @ x + b1          [C4, N]
    phi   = W2 @ x + b2          [C4, N]
    g     = W3 @ x + b3          [C4, N]
    S     = theta^T @ phi        [N, N]
    A     = softmax(S, axis=-1)
    attn_g[c,i] = sum_j g[c,j] A[i,j]
    y     = x + W4 @ attn_g + b4

Sharding: data-parallel over batch B=8 across the 8 NeuronCores (one batch
element per core).

Per-core algorithm (three-engine balance: PE matmuls / ACT exp / DVE exp):
  - All matmul operands are 16-bit (x/theta/phi/w in fp16, P/g in bf16):
    16-bit weights get a prefetchable background LDWEIGHTS, so matmuls
    stream at ~1 cyc/row with no per-instruction fp32 weight-load stall
    (measured ~130ns/MM for fp32-family weights).
  - S is computed TRANSPOSED: S^T tile [j=128, i=512] = phi_jblk.T @ theta_i
    so that softmax normalization and the PV matmul need no transposes:
      P^T = exp(S^T)  (no max-subtraction: |S| <= ~65 < 88, safe)
      pv[c,i] = sum_j gT[j,c] P^T[j,i]  via matmul with lhsT = [gT | ones]
    The appended ones column makes pv row 64 the softmax denominators l[i].
  - exp is split across TWO engines: ACT does exact exp (bf16 out) for most
    batches; the DVE handles a tunable subset via a one-op Schraudolph
    fast-exp — bf16_bits(exp(x)) ~= int16((2^7/ln2)*x + 127*128 - c) — i.e.
    a single tensor_scalar(mult, add) with int16 output bit-cast to bf16.
    Softmax ratio errors from the +-1.5% approx largely cancel; measured
    end-to-end rel err ~1e-2 vs the 2e-2 gate.
  - attn_g = pv[0:64] * (1/l) broadcast via a K=1 ones matmul.
  - y = x + W4 @ attn_g + b4 fused in one DVE op per tile.
  - Phase A (conv1x1 theta/phi/gT) is emitted interleaved with i-tile 0 of
    the main loop, chunk by chunk, so every engine queue stays in
    dependency order while work chases the x DMA stream.  theta/phi PSUM
    evacuation runs on ACT (Identity + per-partition bias, same table set
    as exp); gT evacuation on DVE.  A memset-fed bf16 warm-up burst un-gates
    the PE HAM clock (1.2 -> 2.4 GHz) before the first x chunk lands.
"""

import sys
from contextlib import ExitStack

import numpy as np

if "/opt/trn_rl_repo" not in sys.path:
    sys.path.insert(0, "/opt/trn_rl_repo")

C = 256
C4 = 64
B = 8
H = W = 64
N = H * W          # 4096
NI = 512           # i-tile width (matmul free dim)
NJ = 128           # j-block (S^T partition dim)
N_ITILES = N // NI   # 8
N_JBLKS = N // NJ    # 32

_CACHE = {}


def _build(cfg):
    import concourse.tile as tile
    from concourse import bacc, mybir

    F32 = mybir.dt.float32
    F16 = mybir.dt.float16

    nc = bacc.Bacc("TRN2", target_bir_lowering=False, debug=False,
                   num_devices=B)

    aps = dict(
        x_d=nc.dram_tensor("x", [128, 2 * N], F16, kind="ExternalInput").ap(),
        w1_d=nc.dram_tensor("w1t", [128, 256], F16, kind="ExternalInput").ap(),
        w2_d=nc.dram_tensor("w2t", [128, 256], F16, kind="ExternalInput").ap(),
        w3_d=nc.dram_tensor("w3t", [128, 128], F16, kind="ExternalInput").ap(),
        w4_d=nc.dram_tensor("w4t", [C4, C], F16, kind="ExternalInput").ap(),
        b123_d=nc.dram_tensor("b123", [128, 3], F32, kind="ExternalInput").ap(),
        b4_d=nc.dram_tensor("b4c", [128, 2], F32, kind="ExternalInput").ap(),
        b3bc_d=nc.dram_tensor("b3bc", [128, C4], F32, kind="ExternalInput").ap(),
        rsc_d=nc.dram_tensor("rscratch", [2, NI], F32, kind="Internal").ap(),
        y_d=nc.dram_tensor("y", [C, N], F32, kind="ExternalOutput").ap(),
    )

    with tile.TileContext(nc) as tc:
        _body(nc, tc, cfg, aps)
    nc.compile()
    return nc


def _body(nc, tc, cfg, aps):
    from concourse import bass as cbass
    from concourse import mybir
    from concourse.alu_op_type import AluOpType as Alu

    F32 = mybir.dt.float32
    F16 = mybir.dt.float16
    BF16 = mybir.dt.bfloat16
    I16 = mybir.dt.int16
    Exp = mybir.ActivationFunctionType.Exp
    Ident = mybir.ActivationFunctionType.Identity

    # Schraudolph fast-exp constants: bf16_bits(e^x) ~= int16(EXPA*x + EXPB)
    EXPA = 184.6650085170266          # 2^7 / ln 2
    EXPB = 16256.5 - cfg.get("expc", 4.7)   # 127*128 + trunc-comp - center
    # Batches where exp runs entirely on ACT (one wide ACTIVATE); these host
    # the previous i-tile's tail chain so the DVE is free for it.  All other
    # batches split their two j-blocks ACROSS the engines (jb0 on ACT, jb1
    # on DVE concurrently) so per-batch exp latency ~0.7us < per-batch PE
    # work — otherwise the S->exp->PV->S psum-slot loop can't close and
    # every engine idles in turn.
    ACT_FULL = set(cfg.get("act_full", (2, 6, 10, 13)))

    x_d, y_d = aps["x_d"], aps["y_d"]

    with ExitStack() as st:
        sb = st.enter_context(tc.tile_pool(name="sb", bufs=1))

        # ---- static SBUF tensors ----
        xr_sb = sb.tile([128, 2 * N], F16, tag="xr_sb")
        # theta/phi duplicated across both partition halves (rows 64-127 =
        # rows 0-63) so S^T matmul pairs can row-pack the full PE array.
        th_t = [sb.tile([128, NI], F16, tag=f"th{n}", name=f"tht{n}")
                for n in range(N_ITILES)]
        ph_t = [sb.tile([128, NI], F16, tag=f"ph{n}", name=f"pht{n}")
                for n in range(N_ITILES)]
        gt_t = [sb.tile([128, 4 * (C4 + 1)], BF16, tag=f"gt{n}",
                        name=f"gtt{n}") for n in range(N_ITILES)]

        def ph_ap(jb):
            return ph_t[jb // 4][:, (jb % 4) * NJ:(jb % 4 + 1) * NJ]

        def gt_ap(jb):
            o = (jb % 4) * (C4 + 1)
            return gt_t[jb // 4][:, o:o + C4 + 1]

        w1_sb = sb.tile([128, 256], F16, tag="w1_sb")       # dup-M k-tiles
        w2_sb = sb.tile([128, 256], F16, tag="w2_sb")
        w3_sb = sb.tile([128, 128], F16, tag="w3_sb")
        w4_sb = sb.tile([C4, C], F16, tag="w4_sb")
        b123_sb = sb.tile([128, 3], F32, tag="b123_sb")
        b4_sb = sb.tile([128, 2], F32, tag="b4_sb")
        ones_sb = sb.tile([1, 128], F32, tag="ones_sb")
        ones_r_sb = sb.tile([1, C4], BF16, tag="ones_r_sb")
        b3bc_sb = sb.tile([128, C4], F32, tag="b3bc_sb")

        # ---- PSUM pools (8 banks total: stage 4 + pv 2 + misc 1 + warm 1).
        # Phase-A conv tiles share the "stage" tag/rotation.
        HALF = 1024                     # cols per staging half (2 banks)
        JPB = HALF // NI                # j-blocks per exp batch
        ps_stage = st.enter_context(
            tc.tile_pool(name="ps_stage", bufs=2, space="PSUM"))
        ps_pv = st.enter_context(
            tc.tile_pool(name="ps_pv", bufs=2, space="PSUM"))
        ps_misc = st.enter_context(
            tc.tile_pool(name="ps_misc", bufs=1, space="PSUM"))
        ps_warm = st.enter_context(
            tc.tile_pool(name="ps_warm", bufs=1, space="PSUM"))
        pt_pool = st.enter_context(tc.tile_pool(name="pt", bufs=4))
        dv_pool = st.enter_context(tc.tile_pool(name="dv", bufs=3))
        y_pool = st.enter_context(tc.tile_pool(name="yp", bufs=4))

        # ---- PE warm-up burst: memset-fed bf16 matmuls with no DMA deps,
        # queued first so the HAM clock gate un-throttles (1.2->2.4GHz)
        # while the x DMA is still in flight.
        warm_w = sb.tile([128, 128], BF16, tag="warm_w")
        warm_x = sb.tile([128, 256], BF16, tag="warm_x")
        nc.vector.memset(warm_w[:], 0.5)
        nc.vector.memset(warm_x[:], 0.5)
        warm_ps = ps_warm.tile([128, 256], F32, tag="warm")
        for r in range(cfg.get("warm", 30)):
            nc.tensor.matmul(warm_ps[:], warm_w[:], warm_x[:],
                             start=True, stop=True)

        def filler():
            # one always-ready matmul: keeps the HAM activity monitor fed
            # through dependency stalls (a >~3.4us PE-idle window throttles
            # the PE clock to 1.2GHz for the next several us)
            nc.tensor.matmul(warm_ps[:], warm_w[:], warm_x[:],
                             start=True, stop=True)

        def filler64():
            # cheap (~60ns) HAM-insurance variant
            nc.tensor.matmul(warm_ps[:, 0:64], warm_w[:], warm_x[:, 0:64],
                             start=True, stop=True)

        # Input DMAs are spread across the three DMA-capable queues (SP
        # hardware DGE, gpsimd software DGE, ACT hardware DGE) and ordered
        # x-chunk-0 FIRST: the real HBM transfer of x takes ~6-8us, so the
        # first conv chunk's data must be at the head of the transfer
        # stream, with the (small, fast) weight transfers interleaved after.
        def x_dma(eng, p):
            for k in range(2):
                c0 = k * N + p * 2 * NI
                eng.dma_start(xr_sb[:, c0:c0 + 2 * NI], x_d[:, c0:c0 + 2 * NI])

        nc.sync.dma_start(w1_sb[:], aps["w1_d"][:])
        nc.sync.dma_start(w2_sb[:], aps["w2_d"][:])
        nc.sync.dma_start(b123_sb[:], aps["b123_d"][:])
        x_dma(nc.sync, 0)
        x_dma(nc.sync, 1)
        # gate the other two queues behind the critical head of the wire
        # (w1/w2/b123 + x chunk 0): the HBM wire is shared, so without the
        # gate their x transfers steal bandwidth and the first conv's data
        # arrives ~7us late.
        gate_sb = sb.tile([128, 3], F32, tag="gate_sb")
        nc.gpsimd.dma_start(gate_sb[:], b123_sb[:])
        nc.gpsimd.dma_start(w3_sb[:], aps["w3_d"][:])
        nc.gpsimd.dma_start(b3bc_sb[:], aps["b3bc_d"][:])
        x_dma(nc.gpsimd, 2)
        nc.scalar.dma_start(w4_sb[:], aps["w4_d"][:])
        nc.scalar.dma_start(b4_sb[:], aps["b4_d"][:])
        gate2_sb = sb.tile([128, 3], F32, tag="gate2_sb")
        nc.scalar.dma_start(gate2_sb[:], b123_sb[:])
        x_dma(nc.scalar, 3)
        nc.vector.memset(ones_sb[:], 1.0)
        nc.vector.tensor_copy(ones_r_sb[:], ones_sb[:, 0:C4])
        ones_col = sb.tile([128, N_JBLKS], F32, tag="ones_col")
        nc.vector.memset(ones_col[:], 1.0)
        for n in range(N_ITILES):
            nc.vector.tensor_copy(
                gt_t[n][:].rearrange("p (j c) -> p j c", c=C4 + 1)
                [:, :, C4:C4 + 1],
                ones_col[:, 4 * n:4 * n + 4]
                .rearrange("p (j c) -> p j c", c=1))

        # ---- phase A: conv1x1 chunks, emitted lazily (interleaved with
        # i-tile 0 of the main loop so each engine queue stays in
        # x-DMA-chunk dependency order).
        def emit_chunk(n):
            # theta/phi: one merged [128, 2*NI] stage slot per chunk (fewer
            # stage-pool allocations — each alloc stalls the PE on the
            # slot's previous exp); PSUM evac on ACT (Identity +
            # per-partition bias — same table set as Exp).
            cps = ps_stage.tile([128, 2 * NI], F32, tag="stage",
                                name=f"cps{n}")
            for half_i, (dst_t, w_sb_, col) in enumerate(
                    ((ph_t, w2_sb, 1), (th_t, w1_sb, 0))):
                ps = cps[:, half_i * NI:(half_i + 1) * NI]
                for k in range(2):
                    nc.tensor.matmul(
                        ps, w_sb_[:, k * 128:(k + 1) * 128],
                        xr_sb[:, k * N + n * NI:k * N + (n + 1) * NI],
                        start=(k == 0), stop=(k == 1))
                nc.scalar.activation(dst_t[n][:], ps, Ident,
                                     bias=b123_sb[:, col:col + 1])
            filler64()
            # gT direct: 4 j-blocks merged into one psum tile; evac on DVE
            tp = ps_stage.tile([128, 4 * C4], F32, tag="stage",
                               name=f"gps{n}")
            for q in range(4):
                nb = 4 * n + q
                for k in range(2):
                    nc.tensor.matmul(
                        tp[:, q * C4:(q + 1) * C4],
                        xr_sb[:, k * N + nb * NJ:k * N + (nb + 1) * NJ],
                        w3_sb[:, k * C4:(k + 1) * C4],
                        start=(k == 0), stop=(k == 1))
            for q in range(4):
                o = (4 * n + q) % 4 * (C4 + 1)
                nc.vector.scalar_tensor_tensor(
                    gt_t[n][:, o:o + C4], tp[:, q * C4:(q + 1) * C4], 1.0,
                    b3bc_sb[:], Alu.mult, Alu.add)

        chunks_done = [0]

        def ensure_chunks(upto):
            while chunks_done[0] <= upto:
                emit_chunk(chunks_done[0])
                chunks_done[0] += 1

        # ---- main loop ----
        def make_batches(i):
            # i == 0: start with single-j-block batches so the first exps
            # arrive quickly while phase A still chases the x DMA.
            sizes = [1, 1] if i == 0 else []
            done = sum(sizes)
            while done < N_JBLKS:
                nb = min(JPB, N_JBLKS - done)
                sizes.append(nb)
                done += nb
            out, j = [], 0
            for s in sizes:
                out.append(list(range(j, j + s)))
                j += s
            return out

        pvs = [None] * N_ITILES

        def emit_s(i, b, batches):
            if i == 0:
                ensure_chunks(max(batches[b]) // 4)
            # row-packed pairs: even j-blocks on PE rows 0-63, odd on 64-127
            # (theta/phi are duplicated across halves).
            if i == 0 and b == 0:
                stage_t = ps_misc.tile([128, NI], F32, tag="misc",
                                       name="stage00")
            else:
                stage_t = ps_stage.tile([128, HALF], F32, tag="stage",
                                        name=f"stage_{i}_{b}")
            half = stage_t[:, 0:len(batches[b]) * NI]
            for k, jb in enumerate(batches[b]):
                lo = (jb % 2) * C4
                nc.tensor.matmul(
                    half[:, k * NI:(k + 1) * NI],
                    ph_ap(jb)[lo:lo + C4, :],
                    th_t[i][lo:lo + C4, :],
                    start=True, stop=True,
                    tile_position=(lo, 0))
            return half

        def emit_exp(i, b, half, batches):
            blist = batches[b]
            w = len(blist) * NI
            pt = pt_pool.tile([128, HALF], BF16, tag="pt")
            if len(blist) == 1:
                # i0 pipeline-fill singles: DVE (ACT is busy with conv evacs)
                nc.vector.tensor_scalar(
                    pt[:, 0:NI].bitcast(I16), half[:, 0:NI],
                    EXPA, EXPB, Alu.mult, Alu.add)
            elif i > 0 and b in ACT_FULL:
                nc.scalar.activation(pt[:, 0:w], half[:], Exp)
            else:
                # split the batch across both engines: exact exp for jb0 on
                # ACT, Schraudolph for jb1 on DVE — concurrently
                nc.scalar.activation(pt[:, 0:NI], half[:, 0:NI], Exp)
                nc.vector.tensor_scalar(
                    pt[:, NI:2 * NI].bitcast(I16), half[:, NI:2 * NI],
                    EXPA, EXPB, Alu.mult, Alu.add)
            return pt

        def emit_pv(i, b, pt, batches):
            pv = pvs[i]
            for k, jb in enumerate(batches[b]):
                nc.tensor.matmul(
                    pv[0:C4 + 1, :],
                    gt_ap(jb),
                    pt[:, k * NI:(k + 1) * NI],
                    start=(jb == 0), stop=(jb == N_JBLKS - 1))

        tail_ag = {}

        def tail_recip(i, dram_bounce=True):
            # 1/l then broadcast [1,NI] -> [C4,NI].  Mid-loop: a gpsimd-DMA
            # DRAM bounce (frees PE+DVE; its ~3us latency hides under the
            # following batches).  Final tail: a K=1 ones-matmul (latency
            # critical).  The l-row copy runs on ACT (it has slack; an extra
            # DVE op here delays the next batches' DVE exp halves).
            pv = pvs[i]
            lrow = dv_pool.tile([1, NI], F32, tag="lrow")
            nc.scalar.activation(lrow[:], pv[C4:C4 + 1, :], Ident)
            recip = dv_pool.tile([1, NI], F32, tag="recip")
            nc.vector.reciprocal_approx_fast(recip[:], lrow[:])
            bcast = dv_pool.tile([C4, NI], F32, tag="bcast")
            if dram_bounce:
                row = aps["rsc_d"][i % 2:i % 2 + 1, :]
                nc.gpsimd.dma_start(row, recip[:])
                bc_src = cbass.AP(tensor=row.tensor, offset=row.offset,
                                  ap=[[0, C4], [1, NI]])
                nc.gpsimd.dma_start(bcast[:], bc_src)
            else:
                recip_r = dv_pool.tile([1, NI], BF16, tag="recip_r")
                nc.vector.tensor_copy(recip_r[:], recip[:])
                bc = ps_misc.tile([128, NI], F32, tag="misc", name=f"bc{i}")
                nc.tensor.matmul(bc[0:C4, :], ones_r_sb[:], recip_r[:],
                                 start=True, stop=True)
                nc.vector.tensor_copy(bcast[:], bc[0:C4, :])
            return bcast

        def tail_ag_mul(i, bcast):
            pv = pvs[i]
            ag = dv_pool.tile([C4, NI], F16, tag="ag")
            nc.vector.tensor_tensor(ag[:], pv[0:C4, :], bcast[:], Alu.mult)
            return ag

        def tail_z(i, ag, h):
            z = ps_misc.tile([128, NI], F32, tag="misc", name=f"z{i}_{h}")
            nc.tensor.matmul(z[:], w4_sb[:, h * 128:(h + 1) * 128],
                             ag[:], start=True, stop=True)
            yt = y_pool.tile([128, NI], F32, tag="yt")
            # y = (z + b4) + x
            nc.vector.scalar_tensor_tensor(
                yt[:], z[:], b4_sb[:, h:h + 1],
                xr_sb[:, h * N + i * NI:h * N + (i + 1) * NI],
                Alu.add, Alu.add)
            nc.sync.dma_start(
                y_d[h * 128:(h + 1) * 128, i * NI:(i + 1) * NI], yt[:])

        # software-pipelined emission, depth 2 on the PE stream: at step t
        # the PE sees [S(t+1), PV(t-1)] — PV consumes exp results that are a
        # full batch old, so it never stalls on the exp engines; S(t+1)
        # reuses the psum slot of S(t-1), whose exp also had a full batch of
        # slack.  The previous i-tile's tail chain is spread over batches
        # 2/5/8/11 of the next i-tile.
        all_batches = {i: make_batches(i) for i in range(N_ITILES)}
        flat = [(i, b) for i in range(N_ITILES)
                for b in range(len(all_batches[i]))]
        halves = {}
        for i in range(N_ITILES):
            pvs[i] = ps_pv.tile([128, NI], F32, tag="pv", name=f"pv{i}")
        halves[flat[0]] = emit_s(*flat[0], all_batches[flat[0][0]])
        pend = []
        for t, (i, b) in enumerate(flat):
            if t + 1 < len(flat):
                ni, nb_ = flat[t + 1]
                halves[flat[t + 1]] = emit_s(ni, nb_, all_batches[ni])
            pt = emit_exp(i, b, halves.pop((i, b)), all_batches[i])
            if i == 0 and b <= 2:
                filler()
            elif b % 2 == 1:
                filler64()
            pend.append((i, b, pt, all_batches[i]))
            if len(pend) > 2:
                emit_pv(*pend.pop(0))
            if i > 0:
                if b == 2:
                    tail_ag["bc"] = tail_recip(i - 1)
                elif b == 6:
                    tail_ag["ag"] = tail_ag_mul(i - 1, tail_ag.pop("bc"))
                elif b == 10:
                    tail_z(i - 1, tail_ag["ag"], 0)
                elif b == 13:
                    tail_z(i - 1, tail_ag.pop("ag"), 1)
        for p in pend:
            emit_pv(*p)
        # final tail: latency-critical (nothing hides it), so broadcast via
        # a ones-matmul instead of the 2-DMA DRAM bounce (~5us of latency),
        # with warm fillers interleaved so the HAM clock gate stays hot
        # through the last serial chain.
        i = N_ITILES - 1
        bcast_last = tail_recip(i, dram_bounce=False)
        filler()
        ag_last = tail_ag_mul(i, bcast_last)
        filler()
        tail_z(i, ag_last, 0)
        filler()
        tail_z(i, ag_last, 1)


def _prepare_core_inputs(x_b, W1, b1, W2, b2, W3, b3, W4, b4):
    def ktile(wT, m):
        # [256, m] -> [128, 2*m] (two k-tiles side by side)
        return np.ascontiguousarray(
            wT.reshape(2, 128, m).transpose(1, 0, 2).reshape(128, 2 * m))

    def dup(wT):
        # duplicate output channels across both halves: [256,64] -> [256,128]
        return np.concatenate([wT, wT], axis=1)

    f16 = np.float16
    z64 = np.zeros(C4, np.float32)
    return {
        "x": np.ascontiguousarray(
            x_b.reshape(2, 128, N).transpose(1, 0, 2)
            .reshape(128, 2 * N)).astype(f16),
        "w1t": ktile(dup(W1.T), 128).astype(f16),
        "w2t": ktile(dup(W2.T), 128).astype(f16),
        "w3t": ktile(W3.T, C4).astype(f16),
        "w4t": np.ascontiguousarray(W4.T).astype(f16),
        "b123": np.ascontiguousarray(
            np.stack([np.r_[b1, b1], np.r_[b2, b2], np.r_[b3, z64]], axis=1)),
        "b4c": np.ascontiguousarray(b4.reshape(2, 128).T),
        "b3bc": np.ascontiguousarray(
            np.broadcast_to(b3.reshape(1, C4), (128, C4)).copy()),
    }


def kernel(x, W1, b1, W2, b2, W3, b3, W4, b4, _trace=False, _cfg=None):
    from concourse import bass_utils

    cfg = dict(_cfg or {})
    key = tuple(sorted(cfg.items()))
    if key not in _CACHE:
        _CACHE[key] = _build(cfg)
    nc = _CACHE[key]

    x = np.asarray(x, dtype=np.float32)
    xf = x.reshape(B, C, N)
    args = [np.asarray(a, dtype=np.float32)
            for a in (W1, b1, W2, b2, W3, b3, W4, b4)]
    in_maps = [_prepare_core_inputs(xf[b], *args) for b in range(B)]
    res = bass_utils.run_bass_kernel_spmd(
        nc, in_maps, core_ids=list(range(B)), trace=_trace)
    out = np.stack([res.results[b]["y"].reshape(C, H, W) for b in range(B)])
    if _trace:
        kernel.last_exec_time_ns = res.exec_time_ns
    return out
